# revision 1
# baseline (speedup 1.0000x reference)
"""Trainium2 Bass kernel for nn_DelayGSCSNN.

Two-layer adaptive-LIF spiking net with learnable input delays, BN (eval),
and a leaky-integrator readout, scanned over T=100 steps.

Strategy (data-parallel over batch, 8 cores, no collectives):
  - each core simulates B/8 = 32 samples; weights replicated in SBUF.
  - on-device layout: neurons on partitions, batch on the free dim, so the
    spike tiles s1T [128, 8*32] / s2T [128, 4*32] are directly the matmul
    moving operands (rhs) for the next step -- no transposes anywhere.
  - host folds BN (eval stats) and the (1-alpha) input scaling into the
    weight matrices; folds the -THRESH*s and -beta_a*s terms into the
    recurrent weight diagonal; tracks adaptation as A = -a/beta_a so each
    layer's state update is 4 fused DVE ops per step.
  - per-channel fractional delays are folded into a lag-grouped input
    projection (one matmul per step over K = n_lags*C + 1 incl. bias row);
    the lag shift itself is a column-offset copy done once in the prologue.
  - readout: acc = sum_t (1-beta^(T-t))/T * s2_t @ W_out.T, accumulated as
    a weighted spike sum (one DVE op/step) and one matmul at the end.
  - weights in bf16 (spikes are exactly representable; matmul accumulates
    in fp32), state updates in fp32 on the vector engine.
"""

import os
import sys

import numpy as np

for _p in ("/opt/trn_rl_repo", "/root/.axon_site/_ro/trn_rl_repo"):
    if os.path.isdir(_p) and _p not in sys.path:
        sys.path.insert(0, _p)

import concourse.bass as bass
import concourse.tile as tile
from concourse import bacc, mybir
from concourse.bass_utils import run_bass_kernel_spmd

import ml_dtypes

F32 = mybir.dt.float32
BF16 = mybir.dt.bfloat16
OP = mybir.AluOpType

B, T, C = 256, 100, 40
H1, H2, O = 1024, 512, 35
THRESH = 1.0
MAX_DELAY = 30
NCORES = 8
BC = B // NCORES  # batch per core = 32

TRACE = False
TMPDIR = None
LAST_RESULT = None

_CACHE = {}


def _uniform(v):
    v = np.asarray(v, np.float64)
    return float(v.flat[0]) if np.ptp(v) == 0 else None


FP8 = os.environ.get("KFP8", "0") == "1"


def _build_bass(n_lags, lags, kin, scal):
    """Build the Bass program. scal: dict of python-float uniform params."""
    WD = mybir.dt.float8e4 if FP8 else BF16
    nc = bacc.Bacc(None, target_bir_lowering=False)

    # DRAM inputs (per-core shapes; host supplies prepared layouts)
    d_lt1 = nc.dram_tensor("lt1", [128, 8 * H1], WD, kind="ExternalInput")
    d_lt2 = nc.dram_tensor("lt2", [128, 8 * H2], WD, kind="ExternalInput")
    d_ltr2 = nc.dram_tensor("ltr2", [128, 4 * H2], WD, kind="ExternalInput")
    d_lto = nc.dram_tensor("lto", [128, 4 * O], F32, kind="ExternalInput")
    d_wcat = nc.dram_tensor("wcat", [128, H1], BF16, kind="ExternalInput")
    d_x = nc.dram_tensor("xcore", [C, T, BC], F32, kind="ExternalInput")
    d_y = nc.dram_tensor("yout", [O, BC], F32, kind="ExternalOutput")

    A1 = scal["al1"]
    R1 = scal["rh1"]
    RBA1 = scal["rh1"] * scal["ba1"]
    A2 = scal["al2"]
    R2 = scal["rh2"]
    RBA2 = scal["rh2"] * scal["ba2"]
    wt = scal["wt"]  # list of T readout weights (1-beta^(T-t))/T

    with tile.TileContext(nc) as tc:
        with (
            tc.tile_pool(name="const", bufs=1) as cpool,
            tc.tile_pool(name="state", bufs=1) as spool,
            tc.tile_pool(name="psum", bufs=2, space="PSUM") as ppool,
            tc.tile_pool(name="pout", bufs=1, space="PSUM") as opool,
        ):
            # --- weights / input staging ---
            lt1 = cpool.tile([128, 8 * H1], WD)
            lt2 = cpool.tile([128, 8 * H2], WD)
            ltr2 = cpool.tile([128, 4 * H2], WD)
            lto = cpool.tile([128, 4 * O], F32)
            wcat = cpool.tile([128, H1], BF16)
            xf32 = cpool.tile([C, T * BC], F32)
            xt = cpool.tile([128, T * BC], BF16)

            nc.sync.dma_start(lt1[:], d_lt1[:])
            nc.sync.dma_start(lt2[:], d_lt2[:])
            nc.sync.dma_start(ltr2[:], d_ltr2[:])
            nc.sync.dma_start(lto[:], d_lto[:])
            nc.sync.dma_start(wcat[:], d_wcat[:])
            nc.sync.dma_start(xf32[:], d_x[:].rearrange("c t b -> c (t b)"))

            # Engine writes must start at a 32-aligned partition: lag blocks
            # live at partitions 0 and 64; the constant-one bias row is carved
            # from a 32-aligned ones fill (lag1's copy overwrites part of it,
            # leaving row `bias_row` = 1.0 with zero rows after it).
            ones_base = 64 if n_lags == 1 else 96
            salt = len(os.environ.get("KSALT", ""))
            if salt:
                # compile-cache salt: harmless extra memset changes the BIR
                # hash so A/B compiler-flag experiments don't hit the cache
                sc = cpool.tile([1, salt], F32)
                nc.vector.memset(sc[:], 0.0)
            nc.vector.memset(xt[:], 0.0)
            nc.vector.memset(xt[ones_base:128, :], 1.0)
            for li, lg in enumerate(lags):
                if lg < T:
                    nc.vector.tensor_copy(
                        xt[64 * li : 64 * li + C, lg * BC : T * BC],
                        xf32[:, 0 : (T - lg) * BC],
                    )

            # --- states ---
            v1 = spool.tile([128, 8 * BC], F32)
            a1n = spool.tile([128, 8 * BC], F32)
            v2 = spool.tile([128, 4 * BC], F32)
            a2n = spool.tile([128, 4 * BC], F32)
            s2w = spool.tile([128, 4 * BC], F32)
            # spike tiles are parity double-buffered: step t writes buffer
            # t%2 and reads buffer (t-1)%2, so the adaptation update (which
            # reads the OLD spikes) can be emitted AFTER the spike write --
            # keeping the DVE FIFO ahead of the spike write short, since s1
            # gates 96 matmuls of the next round.
            s1b = [spool.tile([128, 8 * BC], BF16, name=f"s1_{i}") for i in range(2)]
            s2b = [spool.tile([128, 4 * BC], BF16, name=f"s2_{i}") for i in range(2)]
            for st in (v1, a1n, v2, a2n, s2w):
                nc.vector.memset(st[:], 0.0)
            for st in s1b + s2b:
                nc.vector.memset(st[:], 0.0)
            if FP8:
                # separate fp8 copies of the spike tiles feed the matmuls
                # (0/1 is exact in fp8); the DVE state update reads bf16.
                s1qb = [spool.tile([128, 8 * BC], WD, name=f"s1q_{i}") for i in range(2)]
                s2qb = [spool.tile([128, 4 * BC], WD, name=f"s2q_{i}") for i in range(2)]
                for st in s1qb + s2qb:
                    nc.vector.memset(st[:], 0.0)
            else:
                s1qb, s2qb = s1b, s2b

            mul, add, sub = OP.mult, OP.add, OP.subtract

            def input_mms(t):
                # input projection into its own PSUM tile as 8 self-contained
                # per-mt groups (start+stop each), so it can be issued one
                # step ahead -- it depends only on xt, and fills the PE stall
                # while the DVE computes the layer-1 spikes.
                pi = ppool.tile([128, 8 * BC], F32, tag="pin")
                for mt in range(8):
                    nc.tensor.matmul(
                        pi[:, mt * BC : (mt + 1) * BC],
                        wcat[0:kin, mt * 128 : (mt + 1) * 128],
                        xt[0:kin, t * BC : (t + 1) * BC],
                        start=True,
                        stop=True,
                    )
                return pi

            pin_next = input_mms(0)

            for t in range(T):
                w, r = t % 2, (t - 1) % 2
                s1, s2 = s1b[w], s2b[w]
                s1o, s2o = s1b[r], s2b[r]
                s1q, s2q = s1qb[w], s2qb[w]
                s1qo, s2qo = s1qb[r], s2qb[r]
                # ---- PE: psum1 = LT1eff @ s1_{t-1} (input part was issued
                # one step ahead into pin_next) ----
                pin = pin_next
                p1 = None
                if t > 0:
                    p1 = ppool.tile([128, 8 * BC], F32, tag="p1")
                    for mt in range(8):
                        po = p1[:, mt * BC : (mt + 1) * BC]
                        for kt in range(8):
                            nc.tensor.matmul(
                                po,
                                lt1[:, kt * H1 + mt * 128 : kt * H1 + (mt + 1) * 128],
                                s1qo[:, kt * BC : (kt + 1) * BC],
                                start=(kt == 0),
                                stop=(kt == 7),
                            )

                p2 = ppool.tile([128, 4 * BC], F32, tag="p2")

                # ---- DVE: layer-1 state update ----
                # v1 = alpha*v1 + pin (runs during rec1) ; v1 += psum1
                # v1 += rho*beta_a*A1neg ; s1 = (v1 >= THRESH)
                # A1neg = rho*A1neg - s1_old (deferred behind the spike write)
                nc.vector.scalar_tensor_tensor(v1[:], v1[:], A1, pin[:], mul, add)
                nc.vector.scalar_tensor_tensor(v1[:], a1n[:], RBA1, v1[:], mul, add)
                if p1 is not None:
                    nc.vector.tensor_add(v1[:], v1[:], p1[:])
                nc.vector.tensor_scalar(s1[:], v1[:], THRESH, None, OP.is_ge)
                if FP8:
                    nc.vector.tensor_scalar(s1q[:], v1[:], THRESH, None, OP.is_ge)
                nc.vector.scalar_tensor_tensor(a1n[:], a1n[:], R1, s1o[:], mul, sub)

                # ---- PE: next step's input projection (independent of s1_t;
                # fills the stall while the DVE computes the spikes) ----
                if t + 1 < T:
                    pin_next = input_mms(t + 1)

                # ---- PE: psum2 = LTr2eff @ s2_{t-1} + LT2eff @ s1_t ----
                # per-mt accumulation groups must be contiguous (one pending
                # group per PSUM tile); rec2 leads so it can start before the
                # DVE finishes s1_t.
                for mt in range(4):
                    po = p2[:, mt * BC : (mt + 1) * BC]
                    if t > 0:
                        for kt in range(4):
                            nc.tensor.matmul(
                                po,
                                ltr2[:, kt * H2 + mt * 128 : kt * H2 + (mt + 1) * 128],
                                s2qo[:, kt * BC : (kt + 1) * BC],
                                start=(kt == 0),
                                stop=False,
                            )
                    for kt in range(8):
                        nc.tensor.matmul(
                            po,
                            lt2[:, kt * H2 + mt * 128 : kt * H2 + (mt + 1) * 128],
                            s1q[:, kt * BC : (kt + 1) * BC],
                            start=(kt == 0 and t == 0),
                            stop=(kt == 7),
                        )

                # ---- DVE: layer-2 state update + readout accumulation ----
                nc.vector.scalar_tensor_tensor(v2[:], v2[:], A2, p2[:], mul, add)
                nc.vector.scalar_tensor_tensor(v2[:], a2n[:], RBA2, v2[:], mul, add)
                nc.vector.tensor_scalar(s2[:], v2[:], THRESH, None, OP.is_ge)
                if FP8:
                    nc.vector.tensor_scalar(s2q[:], v2[:], THRESH, None, OP.is_ge)
                nc.vector.scalar_tensor_tensor(a2n[:], a2n[:], R2, s2o[:], mul, sub)
                nc.vector.scalar_tensor_tensor(s2w[:], s2[:], wt[t], s2w[:], mul, add)

            # ---- epilogue: acc^T = W_out @ s2w  -> DMA out ----
            pO = opool.tile([O, BC], F32)
            # matmul needs matching dtypes; s2w is fp32, lto fp32 (4 cyc/row,
            # only 4 small matmuls).
            for kt in range(4):
                nc.tensor.matmul(
                    pO[:],
                    lto[:, kt * O : (kt + 1) * O],
                    s2w[:, kt * BC : (kt + 1) * BC],
                    start=(kt == 0),
                    stop=(kt == 3),
                )
            yt = spool.tile([O, BC], F32)
            nc.vector.tensor_copy(yt[:], pO[:])
            nc.sync.dma_start(d_y[:], yt[:])

    nc.finalize()
    return nc


def _prepare(inputs):
    """Host-side folding of BN, scalings, delays into device weight layouts."""
    f32 = np.float32
    g1 = inputs["bn1_gamma"] / np.sqrt(inputs["bn1_var"] + 1e-5)
    b1 = inputs["bn1_beta"] - inputs["bn1_mean"] * g1
    g2 = inputs["bn2_gamma"] / np.sqrt(inputs["bn2_var"] + 1e-5)
    b2 = inputs["bn2_beta"] - inputs["bn2_mean"] * g2
    al1, rh1, ba1 = inputs["alpha1"], inputs["rho1"], inputs["beta_a1"]
    al2, rh2, ba2 = inputs["alpha2"], inputs["rho2"], inputs["beta_a2"]
    bo = inputs["beta_out"]

    scal = {}
    for k, v in (("al1", al1), ("rh1", rh1), ("ba1", ba1),
                 ("al2", al2), ("rh2", rh2), ("ba2", ba2), ("bo", bo)):
        u = _uniform(v)
        assert u is not None, f"non-uniform {k} not supported by this kernel"
        scal[k] = u
    scal["wt"] = [float((1.0 - scal["bo"] ** (T - t)) / T) for t in range(T)]

    def to_bf16(a):
        return np.ascontiguousarray(a.astype(ml_dtypes.bfloat16))

    wd_np = mybir.dt.np(mybir.dt.float8e4) if FP8 else ml_dtypes.bfloat16

    def to_wd(a):
        return np.ascontiguousarray(a.astype(wd_np))

    def fold_ktiles(w_eff, nk, m):
        # w_eff: [M, K] effective weight; return lhsT layout [128, nk*m]
        lt = np.ascontiguousarray(w_eff.T)  # [K, M]
        lt = lt.reshape(nk, 128, m).transpose(1, 0, 2).reshape(128, nk * m)
        return lt

    # layer 1 recurrent: fold (1-al)*g and the -(THRESH + beta_a)*I diagonal
    w1 = ((1 - al1) * g1)[:, None].astype(f32) * inputs["W_rec1"]
    w1[np.arange(H1), np.arange(H1)] -= (THRESH + ba1).astype(f32)
    lt1 = to_wd(fold_ktiles(w1, 8, H1))

    w2 = ((1 - al2) * g2)[:, None].astype(f32) * inputs["W2"]
    lt2 = to_wd(fold_ktiles(w2, 8, H2))

    wr2 = ((1 - al2) * g2)[:, None].astype(f32) * inputs["W_rec2"]
    wr2[np.arange(H2), np.arange(H2)] -= (THRESH + ba2).astype(f32)
    ltr2 = to_wd(fold_ktiles(wr2, 4, H2))

    lto = np.ascontiguousarray(
        fold_ktiles(inputs["W_out"].astype(f32), 4, O).astype(f32)
    )

    # input projection with per-channel fractional delays folded into lags
    d = 1.0 / (1.0 + np.exp(-inputs["delay_raw"].astype(np.float64))) * MAX_DELAY
    fl = np.floor(d).astype(np.int64)
    frac = (d - fl).astype(f32)
    lags = sorted(set(fl.tolist()) | set((fl + 1).tolist()))
    n_lags = len(lags)
    assert n_lags <= 2, f"too many distinct delay lags ({n_lags}) for one k-tile"
    # lag block li sits at partitions [64*li, 64*li + C); bias row just after
    # the last lag block, aligned with the device-side ones fill.
    bias_row = 64 if n_lags == 1 else 64 + C
    kin = bias_row + 1
    wd_eff = ((1 - al1) * g1)[:, None].astype(f32) * inputs["W_delay"]  # [H1, C]
    wcat = np.zeros((128, H1), f32)
    for li, lg in enumerate(lags):
        coef = np.where(fl == lg, 1.0 - frac, np.where(fl + 1 == lg, frac, 0.0))
        wcat[64 * li : 64 * li + C, :] = (coef[:, None] * wd_eff.T).astype(f32)
    wcat[bias_row, :] = ((1 - al1) * b1).astype(f32)
    # layer-2 BN bias must be zero for this folding (no per-step bias matmul)
    c2 = (1 - al2) * b2
    assert np.abs(c2).max() == 0.0, "nonzero layer-2 bias not supported"
    wcat_bf = to_bf16(wcat)

    xp = inputs["x"].astype(f32)  # [B, T, C]
    xcores = [
        np.ascontiguousarray(xp[i * BC : (i + 1) * BC].transpose(2, 1, 0))
        for i in range(NCORES)
    ]  # [C, T, BC] each

    common = dict(lt1=lt1, lt2=lt2, ltr2=ltr2, lto=lto, wcat=wcat_bf)
    in_maps = [dict(common, xcore=xcores[i]) for i in range(NCORES)]
    return n_lags, lags, kin, scal, in_maps


def kernel(**inputs):
    global LAST_RESULT
    inputs = {k: np.asarray(v) for k, v in inputs.items()}
    n_lags, lags, kin, scal, in_maps = _prepare(inputs)

    key = (
        tuple(lags),
        kin,
        tuple((k, v) for k, v in sorted(scal.items()) if k != "wt"),
        tuple(scal["wt"]),
    )
    nc = _CACHE.get(key)
    if nc is None:
        nc = _build_bass(n_lags, lags, kin, scal)
        _CACHE[key] = nc

    kw = {}
    if TRACE and TMPDIR:
        os.makedirs(TMPDIR, exist_ok=True)
        kw["tmpdir"] = TMPDIR
    res = run_bass_kernel_spmd(
        nc, in_maps, core_ids=list(range(NCORES)), trace=TRACE, **kw
    )
    LAST_RESULT = res

    out = np.empty((B, O), np.float32)
    for i in range(NCORES):
        out[i * BC : (i + 1) * BC] = res.results[i]["yout"].T
    return out



# revision 3
# speedup vs baseline: 1.2538x; 1.2538x over previous
"""Trainium2 Bass kernel for nn_DelayGSCSNN.

Two-layer adaptive-LIF spiking net with learnable input delays, BN (eval),
and a leaky-integrator readout, scanned over T=100 steps.

Strategy (data-parallel over batch, 8 cores, no collectives):
  - each core simulates B/8 = 32 samples; weights replicated in SBUF.
  - on-device layout: neurons on partitions, batch on the free dim, so the
    spike tiles s1T [128, 8*32] / s2T [128, 4*32] are directly the matmul
    moving operands (rhs) for the next step -- no transposes anywhere.
  - host folds BN (eval stats) and the (1-alpha) input scaling into the
    weight matrices; folds the -THRESH*s and -beta_a*s terms into the
    recurrent weight diagonal; tracks adaptation as A = -a/beta_a so each
    layer's state update is 4 fused DVE ops per step.
  - per-channel fractional delays are folded into a lag-grouped input
    projection (one matmul per step over K = n_lags*C + 1 incl. bias row);
    the lag shift itself is a column-offset copy done once in the prologue.
  - readout: acc = sum_t (1-beta^(T-t))/T * s2_t @ W_out.T, accumulated as
    a weighted spike sum (one DVE op/step) and one matmul at the end.
  - weights in bf16 (spikes are exactly representable; matmul accumulates
    in fp32), state updates in fp32 on the vector engine.
"""

import os
import sys

import numpy as np

for _p in ("/opt/trn_rl_repo", "/root/.axon_site/_ro/trn_rl_repo"):
    if os.path.isdir(_p) and _p not in sys.path:
        sys.path.insert(0, _p)

import concourse.bass as bass
import concourse.tile as tile
from concourse import bacc, mybir
from concourse.bass_utils import run_bass_kernel_spmd

import ml_dtypes

F32 = mybir.dt.float32
BF16 = mybir.dt.bfloat16
OP = mybir.AluOpType

B, T, C = 256, 100, 40
H1, H2, O = 1024, 512, 35
THRESH = 1.0
MAX_DELAY = 30
NCORES = 8
BC = B // NCORES  # batch per core = 32

TRACE = False
TMPDIR = None
LAST_RESULT = None

_CACHE = {}


def _uniform(v):
    v = np.asarray(v, np.float64)
    return float(v.flat[0]) if np.ptp(v) == 0 else None


FP8 = os.environ.get("KFP8", "0") == "1"
STRIP = os.environ.get("KSTRIP", "1") == "1"


def _strip_dead_sem_incs(nc):
    """Drop per-instruction engine-semaphore increments nobody waits on.

    Tile emits a +1 on the engine's counting sem for EVERY instruction so
    any tick is waitable; on HW the increments serialize at ~26-35 ns each
    (slower than the ~26 ns matmul issue rate), so the counter lags the
    engine by ~0.5-1.1 us at the end of a 64-matmul group and every
    cross-engine handoff from the PE eats that lag.  Only ~5 ticks per
    step are actually waited on.  For each pure counting sem (single
    engine, all +1 immediate, all waits ge-imm) keep exactly the
    increments that first reach a waited value and renumber the waits by
    rank; the satisfying instruction for every wait is unchanged, so the
    schedule semantics are identical and no new deadlock is possible.
    """
    from collections import defaultdict

    blocks = nc.main_func.blocks
    upd_n = defaultdict(int)
    upd_eng = defaultdict(set)
    upd_modes = defaultdict(set)
    wait_vals = defaultdict(set)
    wait_bad = defaultdict(bool)
    for b in blocks:
        for inst in b.instructions:
            si = inst.sync_info
            if not si:
                continue
            for u in si.on_update:
                k = (u.id, u.ant_name)
                upd_n[k] += 1
                upd_eng[k].add(str(inst.engine))
                upd_modes[k].add(
                    (u.update_mode, u.update_value, u.update_reg is not None)
                )
            for w in si.on_wait:
                k = (w.id, w.ant_name)
                if w.wait_mode != "sem-ge-imm" or w.wait_reg is not None:
                    wait_bad[k] = True
                else:
                    wait_vals[k].add(w.wait_value)

    victims = {}
    for k in upd_n:
        if len(upd_eng[k]) != 1 or upd_modes[k] != {("sem-inc", 1, False)}:
            continue
        if wait_bad[k]:
            continue
        vals = sorted(v for v in wait_vals.get(k, ()) if v >= 1)
        if any(v > upd_n[k] for v in vals):
            continue
        victims[k] = (set(vals), {v: i + 1 for i, v in enumerate(vals)})

    cnt = defaultdict(int)
    removed = 0
    for b in blocks:
        for inst in b.instructions:
            si = inst.sync_info
            if not si:
                continue
            for w in si.on_wait:
                k = (w.id, w.ant_name)
                if k in victims and w.wait_value >= 1:
                    w.wait_value = victims[k][1][w.wait_value]
            if any((u.id, u.ant_name) in victims for u in si.on_update):
                newu = []
                for u in si.on_update:
                    k = (u.id, u.ant_name)
                    if k in victims:
                        cnt[k] += 1
                        if cnt[k] in victims[k][0]:
                            newu.append(u)
                        else:
                            removed += 1
                    else:
                        newu.append(u)
                if len(newu) != len(si.on_update):
                    si.on_update = newu
    return removed


def _build_bass(n_lags, lags, kin, scal):
    """Build the Bass program. scal: dict of python-float uniform params."""
    WD = mybir.dt.float8e4 if FP8 else BF16
    nc = bacc.Bacc(None, target_bir_lowering=False)

    # DRAM inputs (per-core shapes; host supplies prepared layouts)
    d_lt1 = nc.dram_tensor("lt1", [128, 8 * H1], WD, kind="ExternalInput")
    d_lt2 = nc.dram_tensor("lt2", [128, 8 * H2], WD, kind="ExternalInput")
    d_ltr2 = nc.dram_tensor("ltr2", [128, 4 * H2], WD, kind="ExternalInput")
    d_lto = nc.dram_tensor("lto", [128, 4 * O], F32, kind="ExternalInput")
    d_wcat = nc.dram_tensor("wcat", [128, H1], BF16, kind="ExternalInput")
    d_x = nc.dram_tensor("xcore", [C, T, BC], F32, kind="ExternalInput")
    d_y = nc.dram_tensor("yout", [O, BC], F32, kind="ExternalOutput")

    A1 = scal["al1"]
    R1 = scal["rh1"]
    RBA1 = scal["rh1"] * scal["ba1"]
    A2 = scal["al2"]
    R2 = scal["rh2"]
    RBA2 = scal["rh2"] * scal["ba2"]
    wt = scal["wt"]  # list of T readout weights (1-beta^(T-t))/T

    with tile.TileContext(nc) as tc:
        with (
            tc.tile_pool(name="const", bufs=1) as cpool,
            tc.tile_pool(name="state", bufs=1) as spool,
            tc.tile_pool(name="psum", bufs=2, space="PSUM") as ppool,
            tc.tile_pool(name="pout", bufs=1, space="PSUM") as opool,
        ):
            # --- weights / input staging ---
            lt1 = cpool.tile([128, 8 * H1], WD)
            lt2 = cpool.tile([128, 8 * H2], WD)
            ltr2 = cpool.tile([128, 4 * H2], WD)
            lto = cpool.tile([128, 4 * O], F32)
            wcat = cpool.tile([128, H1], BF16)
            xf32 = cpool.tile([C, T * BC], F32)
            xt = cpool.tile([128, T * BC], BF16)

            nc.sync.dma_start(lt1[:], d_lt1[:])
            nc.sync.dma_start(lt2[:], d_lt2[:])
            nc.sync.dma_start(ltr2[:], d_ltr2[:])
            nc.sync.dma_start(lto[:], d_lto[:])
            nc.sync.dma_start(wcat[:], d_wcat[:])
            nc.sync.dma_start(xf32[:], d_x[:].rearrange("c t b -> c (t b)"))

            # Engine writes must start at a 32-aligned partition: lag blocks
            # live at partitions 0 and 64; the constant-one bias row is carved
            # from a 32-aligned ones fill (lag1's copy overwrites part of it,
            # leaving row `bias_row` = 1.0 with zero rows after it).
            ones_base = 64 if n_lags == 1 else 96
            salt = len(os.environ.get("KSALT", ""))
            if salt:
                # compile-cache salt: harmless extra memset changes the BIR
                # hash so A/B compiler-flag experiments don't hit the cache
                sc = cpool.tile([1, salt], F32)
                nc.vector.memset(sc[:], 0.0)
            nc.vector.memset(xt[:], 0.0)
            nc.vector.memset(xt[ones_base:128, :], 1.0)
            for li, lg in enumerate(lags):
                if lg < T:
                    nc.vector.tensor_copy(
                        xt[64 * li : 64 * li + C, lg * BC : T * BC],
                        xf32[:, 0 : (T - lg) * BC],
                    )

            # --- states ---
            v1 = spool.tile([128, 8 * BC], F32)
            a1n = spool.tile([128, 8 * BC], F32)
            v2 = spool.tile([128, 4 * BC], F32)
            a2n = spool.tile([128, 4 * BC], F32)
            s2w = spool.tile([128, 4 * BC], F32)
            # spike tiles are parity double-buffered: step t writes buffer
            # t%2 and reads buffer (t-1)%2, so the adaptation update (which
            # reads the OLD spikes) can be emitted AFTER the spike write --
            # keeping the DVE FIFO ahead of the spike write short, since s1
            # gates 96 matmuls of the next round.
            s1b = [spool.tile([128, 8 * BC], BF16, name=f"s1_{i}") for i in range(2)]
            s2b = [spool.tile([128, 4 * BC], BF16, name=f"s2_{i}") for i in range(2)]
            for st in (v1, a1n, v2, a2n, s2w):
                nc.vector.memset(st[:], 0.0)
            for st in s1b + s2b:
                nc.vector.memset(st[:], 0.0)
            if FP8:
                # separate fp8 copies of the spike tiles feed the matmuls
                # (0/1 is exact in fp8); the DVE state update reads bf16.
                s1qb = [spool.tile([128, 8 * BC], WD, name=f"s1q_{i}") for i in range(2)]
                s2qb = [spool.tile([128, 4 * BC], WD, name=f"s2q_{i}") for i in range(2)]
                for st in s1qb + s2qb:
                    nc.vector.memset(st[:], 0.0)
            else:
                s1qb, s2qb = s1b, s2b

            mul, add, sub = OP.mult, OP.add, OP.subtract

            def input_mms(t):
                # input projection into its own PSUM tile as 8 self-contained
                # per-mt groups (start+stop each), so it can be issued one
                # step ahead -- it depends only on xt, and fills the PE stall
                # while the DVE computes the layer-1 spikes.
                pi = ppool.tile([128, 8 * BC], F32, tag="pin")
                for mt in range(8):
                    nc.tensor.matmul(
                        pi[:, mt * BC : (mt + 1) * BC],
                        wcat[0:kin, mt * 128 : (mt + 1) * 128],
                        xt[0:kin, t * BC : (t + 1) * BC],
                        start=True,
                        stop=True,
                    )
                return pi

            pin_next = input_mms(0)

            for t in range(T):
                w, r = t % 2, (t - 1) % 2
                s1, s2 = s1b[w], s2b[w]
                s1o, s2o = s1b[r], s2b[r]
                s1q, s2q = s1qb[w], s2qb[w]
                s1qo, s2qo = s1qb[r], s2qb[r]
                # ---- PE: psum1 = LT1eff @ s1_{t-1} (input part was issued
                # one step ahead into pin_next) ----
                pin = pin_next
                p1 = None
                if t > 0:
                    p1 = ppool.tile([128, 8 * BC], F32, tag="p1")
                    for mt in range(8):
                        po = p1[:, mt * BC : (mt + 1) * BC]
                        for kt in range(8):
                            nc.tensor.matmul(
                                po,
                                lt1[:, kt * H1 + mt * 128 : kt * H1 + (mt + 1) * 128],
                                s1qo[:, kt * BC : (kt + 1) * BC],
                                start=(kt == 0),
                                stop=(kt == 7),
                            )

                p2 = ppool.tile([128, 4 * BC], F32, tag="p2")

                # ---- DVE: layer-1 state update ----
                # v1 = alpha*v1 + pin (runs during rec1) ; v1 += psum1
                # v1 += rho*beta_a*A1neg ; s1 = (v1 >= THRESH)
                # A1neg = rho*A1neg - s1_old (deferred behind the spike write)
                nc.vector.scalar_tensor_tensor(v1[:], v1[:], A1, pin[:], mul, add)
                nc.vector.scalar_tensor_tensor(v1[:], a1n[:], RBA1, v1[:], mul, add)
                if p1 is not None:
                    nc.vector.tensor_add(v1[:], v1[:], p1[:])
                nc.vector.tensor_scalar(s1[:], v1[:], THRESH, None, OP.is_ge)
                if FP8:
                    nc.vector.tensor_scalar(s1q[:], v1[:], THRESH, None, OP.is_ge)
                nc.vector.scalar_tensor_tensor(a1n[:], a1n[:], R1, s1o[:], mul, sub)

                # ---- PE: next step's input projection (independent of s1_t;
                # fills the stall while the DVE computes the spikes) ----
                if t + 1 < T:
                    pin_next = input_mms(t + 1)

                # ---- PE: psum2 = LTr2eff @ s2_{t-1} + LT2eff @ s1_t ----
                # per-mt accumulation groups must be contiguous (one pending
                # group per PSUM tile); rec2 leads so it can start before the
                # DVE finishes s1_t.
                for mt in range(4):
                    po = p2[:, mt * BC : (mt + 1) * BC]
                    if t > 0:
                        for kt in range(4):
                            nc.tensor.matmul(
                                po,
                                ltr2[:, kt * H2 + mt * 128 : kt * H2 + (mt + 1) * 128],
                                s2qo[:, kt * BC : (kt + 1) * BC],
                                start=(kt == 0),
                                stop=False,
                            )
                    for kt in range(8):
                        nc.tensor.matmul(
                            po,
                            lt2[:, kt * H2 + mt * 128 : kt * H2 + (mt + 1) * 128],
                            s1q[:, kt * BC : (kt + 1) * BC],
                            start=(kt == 0 and t == 0),
                            stop=(kt == 7),
                        )

                # ---- DVE: layer-2 state update + readout accumulation ----
                nc.vector.scalar_tensor_tensor(v2[:], v2[:], A2, p2[:], mul, add)
                nc.vector.scalar_tensor_tensor(v2[:], a2n[:], RBA2, v2[:], mul, add)
                nc.vector.tensor_scalar(s2[:], v2[:], THRESH, None, OP.is_ge)
                if FP8:
                    nc.vector.tensor_scalar(s2q[:], v2[:], THRESH, None, OP.is_ge)
                nc.vector.scalar_tensor_tensor(a2n[:], a2n[:], R2, s2o[:], mul, sub)
                nc.vector.scalar_tensor_tensor(s2w[:], s2[:], wt[t], s2w[:], mul, add)

            # ---- epilogue: acc^T = W_out @ s2w  -> DMA out ----
            pO = opool.tile([O, BC], F32)
            # matmul needs matching dtypes; s2w is fp32, lto fp32 (4 cyc/row,
            # only 4 small matmuls).
            for kt in range(4):
                nc.tensor.matmul(
                    pO[:],
                    lto[:, kt * O : (kt + 1) * O],
                    s2w[:, kt * BC : (kt + 1) * BC],
                    start=(kt == 0),
                    stop=(kt == 3),
                )
            yt = spool.tile([O, BC], F32)
            nc.vector.tensor_copy(yt[:], pO[:])
            nc.sync.dma_start(d_y[:], yt[:])

    if STRIP:
        _strip_dead_sem_incs(nc)
    nc.finalize()
    return nc


def _prepare(inputs):
    """Host-side folding of BN, scalings, delays into device weight layouts."""
    f32 = np.float32
    g1 = inputs["bn1_gamma"] / np.sqrt(inputs["bn1_var"] + 1e-5)
    b1 = inputs["bn1_beta"] - inputs["bn1_mean"] * g1
    g2 = inputs["bn2_gamma"] / np.sqrt(inputs["bn2_var"] + 1e-5)
    b2 = inputs["bn2_beta"] - inputs["bn2_mean"] * g2
    al1, rh1, ba1 = inputs["alpha1"], inputs["rho1"], inputs["beta_a1"]
    al2, rh2, ba2 = inputs["alpha2"], inputs["rho2"], inputs["beta_a2"]
    bo = inputs["beta_out"]

    scal = {}
    for k, v in (("al1", al1), ("rh1", rh1), ("ba1", ba1),
                 ("al2", al2), ("rh2", rh2), ("ba2", ba2), ("bo", bo)):
        u = _uniform(v)
        assert u is not None, f"non-uniform {k} not supported by this kernel"
        scal[k] = u
    scal["wt"] = [float((1.0 - scal["bo"] ** (T - t)) / T) for t in range(T)]

    def to_bf16(a):
        return np.ascontiguousarray(a.astype(ml_dtypes.bfloat16))

    wd_np = mybir.dt.np(mybir.dt.float8e4) if FP8 else ml_dtypes.bfloat16

    def to_wd(a):
        return np.ascontiguousarray(a.astype(wd_np))

    def fold_ktiles(w_eff, nk, m):
        # w_eff: [M, K] effective weight; return lhsT layout [128, nk*m]
        lt = np.ascontiguousarray(w_eff.T)  # [K, M]
        lt = lt.reshape(nk, 128, m).transpose(1, 0, 2).reshape(128, nk * m)
        return lt

    # layer 1 recurrent: fold (1-al)*g and the -(THRESH + beta_a)*I diagonal
    w1 = ((1 - al1) * g1)[:, None].astype(f32) * inputs["W_rec1"]
    w1[np.arange(H1), np.arange(H1)] -= (THRESH + ba1).astype(f32)
    lt1 = to_wd(fold_ktiles(w1, 8, H1))

    w2 = ((1 - al2) * g2)[:, None].astype(f32) * inputs["W2"]
    lt2 = to_wd(fold_ktiles(w2, 8, H2))

    wr2 = ((1 - al2) * g2)[:, None].astype(f32) * inputs["W_rec2"]
    wr2[np.arange(H2), np.arange(H2)] -= (THRESH + ba2).astype(f32)
    ltr2 = to_wd(fold_ktiles(wr2, 4, H2))

    lto = np.ascontiguousarray(
        fold_ktiles(inputs["W_out"].astype(f32), 4, O).astype(f32)
    )

    # input projection with per-channel fractional delays folded into lags
    d = 1.0 / (1.0 + np.exp(-inputs["delay_raw"].astype(np.float64))) * MAX_DELAY
    fl = np.floor(d).astype(np.int64)
    frac = (d - fl).astype(f32)
    lags = sorted(set(fl.tolist()) | set((fl + 1).tolist()))
    n_lags = len(lags)
    assert n_lags <= 2, f"too many distinct delay lags ({n_lags}) for one k-tile"
    # lag block li sits at partitions [64*li, 64*li + C); bias row just after
    # the last lag block, aligned with the device-side ones fill.
    bias_row = 64 if n_lags == 1 else 64 + C
    kin = bias_row + 1
    wd_eff = ((1 - al1) * g1)[:, None].astype(f32) * inputs["W_delay"]  # [H1, C]
    wcat = np.zeros((128, H1), f32)
    for li, lg in enumerate(lags):
        coef = np.where(fl == lg, 1.0 - frac, np.where(fl + 1 == lg, frac, 0.0))
        wcat[64 * li : 64 * li + C, :] = (coef[:, None] * wd_eff.T).astype(f32)
    wcat[bias_row, :] = ((1 - al1) * b1).astype(f32)
    # layer-2 BN bias must be zero for this folding (no per-step bias matmul)
    c2 = (1 - al2) * b2
    assert np.abs(c2).max() == 0.0, "nonzero layer-2 bias not supported"
    wcat_bf = to_bf16(wcat)

    xp = inputs["x"].astype(f32)  # [B, T, C]
    xcores = [
        np.ascontiguousarray(xp[i * BC : (i + 1) * BC].transpose(2, 1, 0))
        for i in range(NCORES)
    ]  # [C, T, BC] each

    common = dict(lt1=lt1, lt2=lt2, ltr2=ltr2, lto=lto, wcat=wcat_bf)
    in_maps = [dict(common, xcore=xcores[i]) for i in range(NCORES)]
    return n_lags, lags, kin, scal, in_maps


def kernel(**inputs):
    global LAST_RESULT
    inputs = {k: np.asarray(v) for k, v in inputs.items()}
    n_lags, lags, kin, scal, in_maps = _prepare(inputs)

    key = (
        tuple(lags),
        kin,
        tuple((k, v) for k, v in sorted(scal.items()) if k != "wt"),
        tuple(scal["wt"]),
    )
    nc = _CACHE.get(key)
    if nc is None:
        nc = _build_bass(n_lags, lags, kin, scal)
        _CACHE[key] = nc

    kw = {}
    if TRACE and TMPDIR:
        os.makedirs(TMPDIR, exist_ok=True)
        kw["tmpdir"] = TMPDIR
    res = run_bass_kernel_spmd(
        nc, in_maps, core_ids=list(range(NCORES)), trace=TRACE, **kw
    )
    LAST_RESULT = res

    out = np.empty((B, O), np.float32)
    for i in range(NCORES):
        out[i * BC : (i + 1) * BC] = res.results[i]["yout"].T
    return out

